# revision 2
# baseline (speedup 1.0000x reference)
"""ColBERT MaxSim retrieval kernel for 8 Trainium2 NeuronCores.

Problem (full shapes):
  query_hidden [64,32,768], doc_hidden [256,180,768], query_mask [64,32],
  doc_punct_mask [256,180], W1 [768,768], b1 [768], W2 [768,128], b2 [128]
  out [64, 256]:
    qe = l2norm(relu(qh@W1+b1)@W2+b2 * qm)        # [64,32,128]
    de = l2norm(relu(dh@W1+b1)@W2+b2 * dm)        # [256,180,128]
    s  = einsum('qih,djh->qidj', qe, de) * dm
    out = s.max(-1).sum(1) / qm.sum(-1, keepdims=True)

Sharding: docs split across the 8 cores (32 docs each); queries are
replicated.  Embarrassingly parallel - no collectives.

Host-side mask compaction (exact, not approximate):
  - masked query tokens contribute exactly 0 (their weight in the final
    per-query sum is qm/qsum = 0), so only unmasked query tokens are
    shipped, padded up to a multiple of 512 with zero rows whose weight
    rows in the indicator matrix are 0.
  - masked doc tokens only contribute the "0" baseline to the per-doc max
    (reference multiplies scores by dm before the max).  Each doc keeps its
    unmasked tokens plus >=1 zero-padded slot (rd=0 -> score exactly 0),
    preserving that baseline.  Docs are re-laid at a fixed stride LDP
    (128, or 192 in the cosmically-unlikely case some doc has >=128
    unmasked tokens; ld=180 bounds it).

Math rearrangement (exactly equivalent up to fp rounding):
  e_masked_normed = e_raw * (mask / max(||e_raw||, eps))  per token
  -> scale de^T columns by rd = dm/max(||e_raw||,eps); the query-side
     factor rq >= 0 commutes with the max over doc tokens, so it is
     applied to the per-(qtok,doc) maxima; query mask and the 1/qm.sum()
     normalizer live in a host-built block-indicator matrix used as the
     lhsT of the final reduction matmul.

All embeddings are produced directly in transposed [E/H on partitions,
tokens free] layout so every matmul contraction lands on the partition
dim with zero on-chip transposes (activations are transposed host-side).
fp32r (full-rate fp32 PE path) is used for all large matmuls.
"""

import os
import sys

import numpy as np

for _p in ("/opt/trn_rl_repo",):
    if _p not in sys.path and os.path.isdir(_p):
        sys.path.append(_p)

import concourse.bass as bass
import concourse.mybir as mybir
import concourse.tile as tile
from concourse.bass_utils import run_bass_kernel_spmd

F32 = mybir.dt.float32
F32R = mybir.dt.float32r

# problem dims
NQ, LQ, ND, LD, H, E = 64, 32, 256, 180, 768, 128
NCORES = 8
QT = NQ * LQ                 # 2048 query tokens total (pre-compaction)
NDC = ND // NCORES           # 32 docs per core
KC = H // 128                # 6 contraction chunks
TW = 512                     # token tile width (queries and docs)
EPS = 1e-12

_CACHE = {}


def _build_module(qtp, ldp, split_waits=True, repeats=1):
    """qtp: padded compacted query-token count (multiple of 512).
    ldp: per-doc token stride after compaction (2*ldp <= 512)."""
    dtp = NDC * ldp              # doc tokens per core
    nc = bass.Bass("TRN2", target_bir_lowering=False, debug=False,
                   num_devices=NCORES)

    ntd = dtp // TW
    dh = nc.dram_tensor("dht", [H, dtp], F32R, kind="ExternalInput").ap()
    qh = nc.dram_tensor("qht", [H, qtp], F32R, kind="ExternalInput").ap()
    w1 = nc.dram_tensor("w1", [H, H], F32R, kind="ExternalInput").ap()
    w2 = nc.dram_tensor("w2", [H, E], F32R, kind="ExternalInput").ap()
    b1 = nc.dram_tensor("b1c", [128, KC], F32, kind="ExternalInput").ap()
    b2 = nc.dram_tensor("b2c", [128, 1], F32, kind="ExternalInput").ap()
    dmr = nc.dram_tensor("dmr", [ntd, TW], F32, kind="ExternalInput").ap()
    id4 = nc.dram_tensor("id4", [4, 4], F32, kind="ExternalInput").ap()
    wind = nc.dram_tensor("wind", [qtp, NQ], F32, kind="ExternalInput").ap()
    out = nc.dram_tensor("out", [NQ, NDC], F32, kind="ExternalOutput").ap()

    with tile.TileContext(nc) as tc:
        for _ in range(repeats):
            _emit(tc, nc, qtp, ldp, dh, qh, w1, w2, b1, b2, dmr, wind, id4,
                  out)
    if split_waits:
        _split_multi_waits(nc)
    return nc


def _split_multi_waits(nc, max_waits=1):
    """This walrus build rejects instructions carrying more than one sync
    wait (e.g. the S3_LW stage of fused 4-byte matmuls, Drain). Hoist extra
    waits into standalone same-engine InstEventSemaphore instructions placed
    immediately before the offender - semantics are identical since each
    engine executes its stream in order."""
    n = 0
    for f in nc.m.functions:
        for bb in f.blocks:
            new = []
            for ins in bb.instructions:
                si = ins.sync_info
                waits = list(si.on_wait) if si is not None and si.on_wait else []
                if len(waits) > max_waits:
                    for sw in waits[:-max_waits]:
                        n += 1
                        new.append(mybir.InstEventSemaphore(
                            name=f"WS-{n}", engine=ins.engine, ins=[], outs=[],
                            sync_info=mybir.SyncInfo(on_wait=[sw], on_update=[])))
                    ins.sync_info = mybir.SyncInfo(
                        on_wait=waits[-max_waits:],
                        on_update=list(si.on_update) if si.on_update else [])
                new.append(ins)
            bb.instructions = new


def _emit(tc, nc, qtp, ldp, dh, qh, w1, w2, b1, b2, dmr, wind, id4, out):
    from contextlib import ExitStack

    dtp = NDC * ldp
    ntd, ntq = dtp // TW, qtp // TW
    nqch = qtp // 128            # 128-token query chunks
    # score-tile width: as many whole docs as fit a 512-wide psum bank
    dpg = 512 // ldp             # docs per score tile (4 @ ldp=128)
    dg = dpg * ldp
    ndg = NDC // dpg
    selw = 2 * ntd - 1

    with ExitStack() as ctx:
        cp = ctx.enter_context(tc.tile_pool(name="consts", bufs=1))
        w1_sb = cp.tile([128, KC, H], F32R, tag="w1sb")
        w2_sb = cp.tile([128, KC, E], F32R, tag="w2sb")
        b1_sb = cp.tile([128, KC], F32, tag="b1sb")
        b2_sb = cp.tile([128, 1], F32, tag="b2sb")
        dmr_sb = cp.tile([ntd, TW], F32, tag="dmrsb")
        wind_sb = cp.tile([128, nqch, NQ], F32, tag="windsb")
        # selector: all zeros except column ntd-1 (all ones); slicing
        # sel[:, ntd-1-t : ntd-1-t+M] -> lhsT whose only non-zero column
        # is t, so the ones-reduction lands in psum row t.
        sel_sb = cp.tile([128, selw], F32R, tag="selsb")
        id_sb = cp.tile([4, 4], F32, tag="idsb")
        ones_row = cp.tile([1, 128], F32R, tag="onesrow")
        deT = cp.tile([128, dtp], F32R, tag="deT")
        qeT = cp.tile([128, qtp], F32R, tag="qeT")
        rq_all = cp.tile([128, nqch], F32, tag="rqall")
        rd_sb = cp.tile([ntd, TW], F32R, tag="rdsb")
        rd_row = cp.tile([1, dtp], F32R, tag="rdrow")
        mq_sb = cp.tile([ntq, TW], F32, tag="mqsb")
        md_sb = cp.tile([ntd, TW], F32, tag="mdsb")
        out_sb = cp.tile([NQ, NDC], F32, tag="outsb")

        nc.sync.dma_start(out=w1_sb[:], in_=w1.rearrange("(k p) h -> p k h", p=128))
        nc.sync.dma_start(out=w2_sb[:], in_=w2.rearrange("(k p) e -> p k e", p=128))
        nc.sync.dma_start(out=b1_sb[:], in_=b1)
        nc.sync.dma_start(out=b2_sb[:], in_=b2)
        nc.sync.dma_start(out=dmr_sb[:], in_=dmr)
        nc.sync.dma_start(out=wind_sb[:], in_=wind.rearrange("(g p) q -> p g q", p=128))
        nc.sync.dma_start(out=id_sb[:], in_=id4)
        # memset can't target f32r; build in f32 scratch and copy (the
        # tensor_copy converts, which satisfies the fp32r rounding rule)
        zsc = cp.tile([128, selw], F32, tag="zsc")
        nc.vector.memset(zsc[:], 0.0)
        nc.vector.memset(zsc[:, ntd - 1:ntd], 1.0)
        nc.vector.tensor_copy(sel_sb[:], zsc[:])
        osc = cp.tile([1, 128], F32, tag="osc")
        nc.vector.memset(osc[:], 1.0)
        nc.vector.tensor_copy(ones_row[:], osc[:])

        io_pool = ctx.enter_context(tc.tile_pool(name="io", bufs=5))
        h1_pool = ctx.enter_context(tc.tile_pool(name="h1", bufs=3))
        sq_pool = ctx.enter_context(tc.tile_pool(name="sq", bufs=2))

        def head_tile(src, t, nt, et_dst, psq_acc):
            """MLP head for one 512-token tile; writes e^T into et_dst
            ([128, TW] slice) and accumulates the per-token sum of squares
            into row t of psq_acc [nt, TW] via a selector matmul."""
            xt = io_pool.tile([128, KC, TW], F32R, tag="xt")
            nc.sync.dma_start(out=xt[:],
                              in_=src.rearrange("(k p) n -> p k n", p=128)
                              [:, :, t * TW:(t + 1) * TW])
            h1 = h1_pool.tile([128, KC, TW], F32R, tag="h1")
            for h in range(KC):
                ph = ph_pool.tile([128, TW], F32, tag="ph")
                for k in range(KC):
                    nc.tensor.matmul(
                        ph[:],
                        w1_sb[:, k, h * 128:(h + 1) * 128],
                        xt[:, k, :],
                        start=(k == 0), stop=(k == KC - 1))
                nc.scalar.activation(h1[:, h, :], ph[:],
                                     mybir.ActivationFunctionType.Relu,
                                     bias=b1_sb[:, h:h + 1])
            pe = pe_pool.tile([128, TW], F32, tag="pe")
            for h in range(KC):
                nc.tensor.matmul(pe[:], w2_sb[:, h, :], h1[:, h, :],
                                 start=(h == 0), stop=(h == KC - 1))
            nc.scalar.activation(et_dst, pe[:],
                                 mybir.ActivationFunctionType.Identity,
                                 bias=b2_sb[:, 0:1])
            sq = sq_pool.tile([128, TW], F32R, tag="sq")
            nc.gpsimd.tensor_mul(sq[:], et_dst, et_dst)
            nc.tensor.matmul(psq_acc, sel_sb[:, ntd - 1 - t:ntd - 1 - t + nt],
                             sq[:], start=(t == 0), stop=(t == nt - 1))

        with ExitStack() as pctx:
            ph_pool = pctx.enter_context(
                tc.tile_pool(name="ph", bufs=2, space="PSUM"))
            pe_pool = pctx.enter_context(
                tc.tile_pool(name="pex", bufs=2, space="PSUM"))
            psq_pool = pctx.enter_context(
                tc.tile_pool(name="psq", bufs=1, space="PSUM"))
            ptr_pool = pctx.enter_context(
                tc.tile_pool(name="ptr", bufs=1, space="PSUM"))

            # ---- docs ----
            psq_d = psq_pool.tile([ntd, TW], F32, tag="psqd")
            for t in range(ntd):
                head_tile(dh, t, ntd, deT[:, t * TW:(t + 1) * TW], psq_d[:])
            # rd = dm / max(sqrt(ssq), eps), in [ntd, TW] layout
            nc.vector.tensor_copy(md_sb[:], psq_d[:])
            nc.scalar.activation(md_sb[:], md_sb[:],
                                 mybir.ActivationFunctionType.Sqrt)
            nc.vector.tensor_scalar_max(md_sb[:], md_sb[:], EPS)
            with nc.allow_low_precision(reason="f32r has ample mantissa "
                                        "for unit-scale norm reciprocals"):
                nc.vector.reciprocal(rd_sb[:], md_sb[:])
                nc.vector.tensor_mul(rd_sb[:], rd_sb[:], dmr_sb[:])
            # re-lay [ntd, TW] -> one [1, dtp] row so every broadcast source
            # sits at partition base 0 (engines can't address base>0)
            nc.sync.dma_start(out=rd_row[:], in_=rd_sb[:])
            # scale deT columns by rd: K=1 ones-row matmul broadcasts each
            # [1, TW] slice across all 128 partitions via PSUM
            for t in range(ntd):
                bc = ph_pool.tile([128, TW], F32, tag="ph")
                nc.tensor.matmul(bc[:], ones_row[:],
                                 rd_row[:, t * TW:(t + 1) * TW],
                                 start=True, stop=True)
                sl = deT[:, t * TW:(t + 1) * TW]
                nc.vector.tensor_mul(sl, sl, bc[:])

            # ---- queries ----
            psq_q = psq_pool.tile([ntq, TW], F32, tag="psqq")
            for t in range(ntq):
                head_tile(qh, t, ntq, qeT[:, t * TW:(t + 1) * TW], psq_q[:])
            # rq = 1 / max(sqrt(ssq), eps)  (query mask folded into wind)
            nc.vector.tensor_copy(mq_sb[:], psq_q[:])
            nc.scalar.activation(mq_sb[:], mq_sb[:],
                                 mybir.ActivationFunctionType.Sqrt)
            nc.vector.tensor_scalar_max(mq_sb[:], mq_sb[:], EPS)
            nc.vector.reciprocal(mq_sb[:], mq_sb[:])
            # transpose [ntq, TW] rows -> rq_all [128, nqch] columns
            rq_v = rq_all[:].rearrange("p (t c) -> p t c", c=4)
            for c in range(4):
                ptr = ptr_pool.tile([128, 4], F32, tag="ptr")
                nc.tensor.transpose(ptr[:, :ntq],
                                    mq_sb[:, c * 128:(c + 1) * 128],
                                    id_sb[0:ntq, 0:ntq])
                nc.vector.tensor_copy(rq_v[:, :, c], ptr[:, :ntq])

        # ---- scores ----
        with ExitStack() as sctx:
            ps_pool = sctx.enter_context(
                tc.tile_pool(name="ps", bufs=6, space="PSUM"))
            po_pool = sctx.enter_context(
                tc.tile_pool(name="po", bufs=1, space="PSUM"))
            m_pool = sctx.enter_context(tc.tile_pool(name="m", bufs=nqch))

            m_tiles = []
            for g in range(nqch):
                qchunk = qeT[:, g * 128:(g + 1) * 128]
                mt = m_pool.tile([128, NDC], F32, tag="mt")
                for j in range(ndg):
                    ps = ps_pool.tile([128, dg], F32, tag="ps")
                    nc.tensor.matmul(ps[:], qchunk,
                                     deT[:, j * dg:(j + 1) * dg],
                                     start=True, stop=True)
                    nc.vector.tensor_reduce(
                        mt[:, j * dpg:(j + 1) * dpg],
                        ps[:].rearrange("p (d j) -> p d j", j=ldp),
                        axis=mybir.AxisListType.X, op=mybir.AluOpType.max)
                nc.vector.tensor_scalar_mul(mt[:], mt[:], rq_all[:, g:g + 1])
                m_tiles.append(mt)

            pout = po_pool.tile([NQ, NDC], F32, tag="pout")
            for g in range(nqch):
                nc.tensor.matmul(pout[:], wind_sb[:, g, :], m_tiles[g][:],
                                 start=(g == 0), stop=(g == nqch - 1))
            nc.vector.tensor_copy(out_sb[:], pout[:])
            nc.sync.dma_start(out=out, in_=out_sb[:])


def _get_module(qtp, ldp):
    key = ("nc", qtp, ldp)
    if key not in _CACHE:
        _CACHE[key] = _build_module(qtp, ldp)
    return _CACHE[key]


def _prep_inputs(query_hidden, doc_hidden, query_mask, doc_punct_mask,
                 W1, b1, W2, b2):
    """Host-side compaction + shard + layout prep.
    Returns (per-core input maps, qtp, ldp)."""
    f32 = np.float32
    qh2 = np.asarray(query_hidden, f32).reshape(QT, H)
    dh2 = np.asarray(doc_hidden, f32).reshape(ND * LD, H)
    qm = np.asarray(query_mask, f32).reshape(QT)
    dmf = np.asarray(doc_punct_mask, f32).reshape(ND, LD)
    w1 = np.ascontiguousarray(np.asarray(W1, f32))
    w2 = np.ascontiguousarray(np.asarray(W2, f32))
    b1c = np.ascontiguousarray(np.asarray(b1, f32).reshape(KC, 128).T)
    b2c = np.ascontiguousarray(np.asarray(b2, f32).reshape(E, 1))

    # ---- query compaction ----
    qidx = np.nonzero(qm > 0)[0]
    kq = len(qidx)
    qtp = max(TW, int(-(-kq // TW)) * TW)
    qh_c = np.zeros((qtp, H), f32)
    qh_c[:kq] = qh2[qidx]
    qht = np.ascontiguousarray(qh_c.T)
    qsum = qm.reshape(NQ, LQ).sum(axis=1)
    qsum = np.maximum(qsum, 1.0)
    wind = np.zeros((qtp, NQ), f32)
    qnum = qidx // LQ
    wind[np.arange(kq), qnum] = 1.0 / qsum[qnum]

    # ---- doc compaction ----
    cnt = (dmf > 0).sum(axis=1)
    # every doc keeps >=1 zero slot for the masked-score baseline;
    # ld=180 guarantees 192 always suffices
    ldp = 128 if cnt.max() < 128 else 192
    dtp = NDC * ldp
    ntd = dtp // TW

    in_maps = []
    for c in range(NCORES):
        dh_c = np.zeros((dtp, H), f32)
        dm_c = np.zeros(dtp, f32)
        for i in range(NDC):
            d = c * NDC + i
            idx = np.nonzero(dmf[d] > 0)[0]
            n = len(idx)
            dh_c[i * ldp:i * ldp + n] = dh2[d * LD + idx]
            dm_c[i * ldp:i * ldp + n] = 1.0
        in_maps.append({
            "dht": np.ascontiguousarray(dh_c.T),
            "qht": qht,
            "w1": w1,
            "w2": w2,
            "b1c": b1c,
            "b2c": b2c,
            "dmr": dm_c.reshape(ntd, TW),
            "id4": np.eye(4, dtype=f32),
            "wind": wind,
        })
    return in_maps, qtp, ldp


def kernel(query_hidden, doc_hidden, query_mask, doc_punct_mask,
           W1, b1, W2, b2):
    in_maps, qtp, ldp = _prep_inputs(query_hidden, doc_hidden, query_mask,
                                     doc_punct_mask, W1, b1, W2, b2)
    nc = _get_module(qtp, ldp)
    res = run_bass_kernel_spmd(nc, in_maps, list(range(NCORES)))
    _CACHE["last_results"] = res
    outs = [np.asarray(res.results[c]["out"]) for c in range(NCORES)]
    return np.concatenate(outs, axis=1).astype(np.float32)



# revision 15
# speedup vs baseline: 1.0721x; 1.0721x over previous
"""ColBERT MaxSim retrieval kernel for 8 Trainium2 NeuronCores.

Problem (full shapes):
  query_hidden [64,32,768], doc_hidden [256,180,768], query_mask [64,32],
  doc_punct_mask [256,180], W1 [768,768], b1 [768], W2 [768,128], b2 [128]
  out [64, 256]:
    qe = l2norm(relu(qh@W1+b1)@W2+b2 * qm)        # [64,32,128]
    de = l2norm(relu(dh@W1+b1)@W2+b2 * dm)        # [256,180,128]
    s  = einsum('qih,djh->qidj', qe, de) * dm
    out = s.max(-1).sum(1) / qm.sum(-1, keepdims=True)

Sharding: docs split across the 8 cores (32 docs each); queries are
replicated.  Embarrassingly parallel - no collectives.

Host-side mask compaction (exact, not approximate):
  - masked query tokens contribute exactly 0 (their weight in the final
    per-query sum is qm/qsum = 0), so only unmasked query tokens are
    shipped, padded up to a multiple of 512 with zero rows whose weight
    rows in the indicator matrix are 0.
  - masked doc tokens only contribute the "0" baseline to the per-doc max
    (reference multiplies scores by dm before the max).  Each doc keeps its
    unmasked tokens plus >=1 zero-padded slot (rd=0 -> score exactly 0),
    preserving that baseline.  Docs are re-laid at a fixed stride LDP
    (128, or 192 in the cosmically-unlikely case some doc has >=128
    unmasked tokens; ld=180 bounds it).

Math rearrangement (exactly equivalent up to fp rounding):
  e_masked_normed = e_raw * (mask / max(||e_raw||, eps))  per token
  -> de^T columns scaled by rd = dm/max(||e_raw||,eps) and qe^T columns
     by rq = 1/max(||e_raw||,eps) (the query mask and the 1/qm.sum()
     normalizer live in a host-built block-indicator matrix used as the
     lhsT of the final reduction matmul; rq >= 0 commutes with the max
     over doc tokens so scaling qe is exactly the reference math).

Pipelined schedule: the MLP head (PE-heavy) for doc-tile group k+1 is
emitted between group k's norm chain (DVE/Act) and group k's score
matmuls + max-reduces (PE+DVE), so the vector-bound score reductions
hide under tensor-engine head work instead of serializing after it.

All embeddings are produced directly in transposed [E/H on partitions,
tokens free] layout so every matmul contraction lands on the partition
dim with zero on-chip transposes (activations are transposed host-side).
fp32r (full-rate fp32 PE path) is used for all large matmuls.
"""

import os
import sys

import numpy as np

for _p in ("/opt/trn_rl_repo",):
    if _p not in sys.path and os.path.isdir(_p):
        sys.path.append(_p)

import concourse.bass as bass
import concourse.mybir as mybir
import concourse.tile as tile
from concourse.bass_utils import run_bass_kernel_spmd

F32 = mybir.dt.float32
F32R = mybir.dt.float32r

# problem dims
NQ, LQ, ND, LD, H, E = 64, 32, 256, 180, 768, 128
NCORES = 8
QT = NQ * LQ                 # 2048 query tokens total (pre-compaction)
NDC = ND // NCORES           # 32 docs per core
KC = H // 128                # 6 contraction chunks
TW = 512                     # token tile width (queries and docs)
GS = 2                       # doc tiles per norm/score pipeline group
EPS = 1e-12

_CACHE = {}


def _build_module(qtp, ldp, nqr, split_waits=True, repeats=1):
    """qtp: padded compacted query-token count (multiple of 512).
    ldp: per-doc token stride after compaction (2*ldp <= 512).
    nqr: number of 128-token query chunks with any unmasked token."""
    dtp = NDC * ldp              # doc tokens per core
    nc = bass.Bass("TRN2", target_bir_lowering=False, debug=False,
                   num_devices=NCORES)

    ntd = dtp // TW
    dh = nc.dram_tensor("dht", [H, dtp], F32R, kind="ExternalInput").ap()
    qh = nc.dram_tensor("qht", [H, qtp], F32R, kind="ExternalInput").ap()
    w1 = nc.dram_tensor("w1", [H, H], F32R, kind="ExternalInput").ap()
    w2 = nc.dram_tensor("w2", [H, E], F32R, kind="ExternalInput").ap()
    b1 = nc.dram_tensor("b1c", [128, KC], F32, kind="ExternalInput").ap()
    b2 = nc.dram_tensor("b2c", [128, 1], F32, kind="ExternalInput").ap()
    dpad = nc.dram_tensor("dpad", [1, dtp], F32R, kind="ExternalInput").ap()
    wind = nc.dram_tensor("wind", [qtp, NQ], F32, kind="ExternalInput").ap()
    out = nc.dram_tensor("out", [NQ, NDC], F32, kind="ExternalOutput").ap()

    with tile.TileContext(nc) as tc:
        for _ in range(repeats):
            _emit(tc, nc, qtp, ldp, nqr, dh, qh, w1, w2, b1, b2, dpad, wind,
                  out)
    if split_waits:
        _split_multi_waits(nc)
    return nc


def _split_multi_waits(nc, max_waits=1):
    """This walrus build rejects instructions carrying more than one sync
    wait (e.g. the S3_LW stage of fused 4-byte matmuls, Drain). Hoist extra
    waits into standalone same-engine InstEventSemaphore instructions placed
    immediately before the offender - semantics are identical since each
    engine executes its stream in order."""
    n = 0
    for f in nc.m.functions:
        for bb in f.blocks:
            new = []
            for ins in bb.instructions:
                si = ins.sync_info
                waits = list(si.on_wait) if si is not None and si.on_wait else []
                if len(waits) > max_waits:
                    for sw in waits[:-max_waits]:
                        n += 1
                        new.append(mybir.InstEventSemaphore(
                            name=f"WS-{n}", engine=ins.engine, ins=[], outs=[],
                            sync_info=mybir.SyncInfo(on_wait=[sw], on_update=[])))
                    ins.sync_info = mybir.SyncInfo(
                        on_wait=waits[-max_waits:],
                        on_update=list(si.on_update) if si.on_update else [])
                new.append(ins)
            bb.instructions = new


def _emit(tc, nc, qtp, ldp, nqr, dh, qh, w1, w2, b1, b2, dpad, wind, out):
    from collections import defaultdict
    from contextlib import ExitStack

    dtp = NDC * ldp
    ntd, ntq = dtp // TW, qtp // TW
    nqch = qtp // 128            # 128-token query chunks
    # score-tile width: as many whole docs as fit a 512-wide psum bank
    dpg = max(1, 512 // ldp)     # docs per score tile (4 @ ldp=112)
    dg = dpg * ldp
    ndg = NDC // dpg
    selw = 2 * ntd - 1

    with ExitStack() as ctx:
        cp = ctx.enter_context(tc.tile_pool(name="consts", bufs=1))
        w1_sb = cp.tile([128, KC, H], F32R, tag="w1sb")
        w2_sb = cp.tile([128, KC, E], F32R, tag="w2sb")
        b1_sb = cp.tile([128, KC], F32, tag="b1sb")
        b2_sb = cp.tile([128, 1], F32, tag="b2sb")
        dpad_sb = cp.tile([1, dtp], F32R, tag="dpadsb")
        wind_sb = cp.tile([128, nqch, NQ], F32, tag="windsb")
        ones_row = cp.tile([1, 128], F32R, tag="onesrow")
        ones_col = cp.tile([128, 1], F32R, tag="onescol")
        deT = cp.tile([128, dtp], F32R, tag="deT")
        qeT = cp.tile([128, qtp], F32R, tag="qeT")
        rd_row = cp.tile([1, dtp], F32R, tag="rdrow")
        rq_row = cp.tile([1, qtp], F32R, tag="rqrow")
        out_sb = cp.tile([NQ, NDC], F32, tag="outsb")

        # w1 split by output-column block so the first W1 matmul only
        # waits on a sixth of the weight traffic
        w1r = w1.rearrange("(k p) h -> p k h", p=128)
        nc.sync.dma_start(out=w1_sb[:, :, 0:128], in_=w1r[:, :, 0:128])

        def dma_consts_rest():
            nc.sync.dma_start(out=b1_sb[:], in_=b1)
            for h in range(1, KC):
                nc.sync.dma_start(out=w1_sb[:, :, h * 128:(h + 1) * 128],
                                  in_=w1r[:, :, h * 128:(h + 1) * 128])
            nc.sync.dma_start(out=w2_sb[:],
                              in_=w2.rearrange("(k p) e -> p k e", p=128))
            nc.sync.dma_start(out=b2_sb[:], in_=b2)
            nc.gpsimd.dma_start(out=dpad_sb[:], in_=dpad)
            nc.gpsimd.dma_start(out=wind_sb[:],
                                in_=wind.rearrange("(g p) q -> p g q", p=128))
        # memset can't target f32r; build in f32 scratch and copy (the
        # tensor_copy converts, which satisfies the fp32r rounding rule)
        osc = cp.tile([128, 128], F32, tag="osc")
        nc.vector.memset(osc[:], 1.0)
        nc.vector.tensor_copy(ones_row[:], osc[0:1, :])
        nc.vector.tensor_copy(ones_col[:], osc[:, 0:1])
        eps_sb = cp.tile([16, 1], F32, tag="epsb")
        nc.vector.memset(eps_sb[:], EPS * EPS)

        io_pool = ctx.enter_context(tc.tile_pool(name="io", bufs=5))
        h1_pool = ctx.enter_context(tc.tile_pool(name="h1", bufs=3))
        sq_pool = ctx.enter_context(tc.tile_pool(name="sq", bufs=2))
        nr_pool = ctx.enter_context(tc.tile_pool(name="nr", bufs=2))
        m_pool = ctx.enter_context(tc.tile_pool(name="m", bufs=nqr))

        ph_pool = ctx.enter_context(
            tc.tile_pool(name="ph", bufs=2, space="PSUM"))
        pe_pool = ctx.enter_context(
            tc.tile_pool(name="pex", bufs=2, space="PSUM"))
        psq_pool = ctx.enter_context(
            tc.tile_pool(name="psq", bufs=1, space="PSUM"))
        ps_pool = ctx.enter_context(
            tc.tile_pool(name="ps", bufs=2, space="PSUM"))
        po_pool = ctx.enter_context(
            tc.tile_pool(name="po", bufs=1, space="PSUM"))

        m_tiles = [m_pool.tile([128, NDC], F32, tag="mt", name=f"mt{g}")
                   for g in range(nqr)]
        pout = po_pool.tile([NQ, NDC], F32, tag="pout")
        box = {}

        # ---------------- emission pieces ----------------

        def dma_xt(T, src, base, w):
            xt = io_pool.tile([128, KC, w], F32R, tag="xt", name=f"xt{T}")
            nc.sync.dma_start(out=xt[:],
                              in_=src.rearrange("(k p) n -> p k n", p=128)
                              [:, :, base:base + w])
            box["xt", T] = xt

        def blk_h(T, h, w):
            """One 128-column block of the W1 matmul + biased ReLU."""
            if h == 0:
                box["h1", T] = h1_pool.tile([128, KC, w], F32R, tag="h1",
                                            name=f"h1_{T}")
            h1 = box["h1", T]
            xt = box["xt", T]
            ph = ph_pool.tile([128, w], F32, tag="ph", name="ph")
            for k in range(KC):
                nc.tensor.matmul(
                    ph[:],
                    w1_sb[:, k, h * 128:(h + 1) * 128],
                    xt[:, k, :],
                    start=(k == 0), stop=(k == KC - 1))
            nc.scalar.activation(h1[:, h, :], ph[:],
                                 mybir.ActivationFunctionType.Relu,
                                 bias=b1_sb[:, h:h + 1])

        def blk_w2(T, et_dst, w):
            """W2 matmul + bias into e^T slice, then squares on Pool."""
            h1 = box["h1", T]
            pe = pe_pool.tile([128, w], F32, tag="pe", name="pe")
            for h in range(KC):
                nc.tensor.matmul(pe[:], w2_sb[:, h, :], h1[:, h, :],
                                 start=(h == 0), stop=(h == KC - 1))
            nc.scalar.activation(et_dst, pe[:],
                                 mybir.ActivationFunctionType.Identity,
                                 bias=b2_sb[:, 0:1])
            sq = sq_pool.tile([128, w], F32R, tag="sq", name=f"sq{T}")
            nc.gpsimd.tensor_mul(sq[:], et_dst, et_dst)
            box["sq", T] = sq

        def psq_mm(T, pad_slice):
            """Deferred per-token sum-of-squares row (ones-column lhsT
            contracts the partition dim): scheduled one slot into the NEXT
            tile so the PE never stalls on the Act->Pool sq chain.  For doc
            tiles a second matmul accumulates the host-built pad row (1e38
            at padded slots) so the norm reciprocal vanishes there - no
            mask multiply anywhere."""
            sq = box["sq", T]
            w = sq.shape[-1]
            psq = psq_pool.tile([1, w], F32, tag="psq", name="psq")
            box["psq", T] = psq
            nc.tensor.matmul(psq[:], ones_col[:], sq[:],
                             start=True, stop=pad_slice is None)
            if pad_slice is not None:
                nc.tensor.matmul(psq[:], ones_col[0:1, 0:1], pad_slice,
                                 start=False, stop=True)

        def norm_chain(T, r_row, base):
            """r = 1/sqrt(ssq + EPS^2) written straight into the [1, TW]
            slice of the scale row - no relay needed at one-tile grain.
            sqrt(x+EPS^2) == max(sqrt(x), EPS) exactly at x=0 and for all
            fp32 x >= EPS^2 (1e-24 vanishes in the mantissa); only tokens
            with 0 < ||e|| < ~1e-12 differ, and those scores are ~1e-12."""
            psq = box["psq", T]
            w = psq.shape[-1]
            md = nr_pool.tile([1, w], F32, tag="md", name="md")
            nc.scalar.activation(md[:], psq[:],
                                 mybir.ActivationFunctionType.Sqrt,
                                 bias=eps_sb[0:1, 0:1])
            with nc.allow_low_precision(reason="f32r has ample mantissa "
                                        "for unit-scale norm reciprocals"):
                nc.vector.reciprocal(r_row[:, base:base + w], md[:])

        def scale_tile(eT, r_row, base, w):
            """Scale eT columns [base, base+w) by r_row: K=1 ones-row
            matmul broadcasts the [1, w] slice across 128 partitions."""
            bc = pe_pool.tile([128, w], F32, tag="pe", name="bc")
            nc.tensor.matmul(bc[:], ones_row[:],
                             r_row[:, base:base + w],
                             start=True, stop=True)
            sl = eT[:, base:base + w]
            nc.vector.tensor_mul(sl, sl, bc[:])

        def score_pair(j, g):
            ps = ps_pool.tile([128, dg], F32, tag="ps", name="ps")
            nc.tensor.matmul(ps[:], qeT[:, g * 128:(g + 1) * 128],
                             deT[:, j * dg:(j + 1) * dg],
                             start=True, stop=True)
            nc.vector.tensor_reduce(
                m_tiles[g][:, j * dpg:(j + 1) * dpg],
                ps[:].rearrange("p (d j) -> p d j", j=ldp),
                axis=mybir.AxisListType.X, op=mybir.AluOpType.max)
            if j == ndg - 1:
                # interleave the output reduction behind the last column
                # block's reduces
                nc.tensor.matmul(pout[:], wind_sb[:, g, :], m_tiles[g][:],
                                 start=(g == 0), stop=(g == nqr - 1))

        # ---------------- static schedule ----------------
        # 7 slots per tile (6 W1 column blocks + W2), ~1.3us of PE work
        # each.  Deferred psq matmuls / norm chains / column scaling /
        # score+reduce pairs are injected into later slots so every
        # engine's queue stays covered by PE head work.
        P = 7
        inject = defaultdict(list)
        seq = [0]

        def sched(slot, pri, fn):
            inject[slot].append((pri, seq[0], fn))
            seq[0] += 1

        # query tiles cover only the nqr scored chunks; the last tile
        # narrows to 256 when that suffices (fp32r full rate needs >= 256)
        tiles = []
        cov = 0
        need = nqr * 128
        while cov < need:
            w = TW if need - cov > 256 else 256
            tiles.append(("q", cov, w))
            cov += w
        assert cov <= qtp
        # narrow tile first: the opening activation DMA halves, so the
        # PE's first matmul starts ~2us earlier
        tiles.sort(key=lambda t: t[2])
        tiles += [("d", c * TW, TW) for c in range(ntd)]
        for T, (kind, base, w) in enumerate(tiles):
            src = qh if kind == "q" else dh
            eT = qeT if kind == "q" else deT
            et = eT[:, base:base + w]
            sched(P * T, 1,
                  lambda T=T, src=src, base=base, w=w: dma_xt(T, src, base, w))
            if T == 0:
                sched(0, 1, dma_consts_rest)
            for h in range(KC):
                sched(P * T + h, 2, lambda T=T, h=h, w=w: blk_h(T, h, w))
            sched(P * T + KC, 2, lambda T=T, et=et, w=w: blk_w2(T, et, w))

        # per-tile psq -> norm -> scale chains; scores fire once their
        # docs' columns are scaled.  The very last tile's chain is
        # emitted immediately (nothing left to cover it anyway).
        fired = 0
        for T, (kind, base, w) in enumerate(tiles):
            if kind == "q":
                r_row, eT, pad = rq_row, qeT, None
            else:
                r_row, eT = rd_row, deT
                pad = dpad_sb[0:1, base:base + w]
            last = T == len(tiles) - 1
            ns = P * T + KC + 1 if last else P * (T + 1) + 1
            sk = 0 if last else 2
            sched(ns, 0, lambda T=T, pad=pad: psq_mm(T, pad))
            sched(ns, 1, lambda T=T, r_row=r_row, base=base:
                  norm_chain(T, r_row, base))
            sched(ns + sk + 1, 3, lambda eT=eT, r_row=r_row, base=base, w=w:
                  scale_tile(eT, r_row, base, w))
            if kind == "d":
                idx = 0
                while fired < ndg and (fired + 1) * dg <= base + w:
                    for g in range(nqr):
                        sched(ns + sk + 2 + idx // 2, 3,
                              lambda j=fired, g=g: score_pair(j, g))
                        idx += 1
                    fired += 1
        assert fired == ndg

        for s in range(max(inject) + 1):
            for _, _, fn in sorted(inject[s], key=lambda x: (x[0], x[1])):
                fn()

        nc.vector.tensor_copy(out_sb[:], pout[:])
        nc.gpsimd.dma_start(out=out, in_=out_sb[:])


def _get_module(qtp, ldp, nqr):
    key = ("nc", qtp, ldp, nqr)
    if key not in _CACHE:
        _CACHE[key] = _build_module(qtp, ldp, nqr)
    return _CACHE[key]


def _prep_inputs(query_hidden, doc_hidden, query_mask, doc_punct_mask,
                 W1, b1, W2, b2):
    """Host-side compaction + shard + layout prep.
    Returns (per-core input maps, qtp, ldp)."""
    f32 = np.float32
    qh2 = np.asarray(query_hidden, f32).reshape(QT, H)
    dh2 = np.asarray(doc_hidden, f32).reshape(ND * LD, H)
    qm = np.asarray(query_mask, f32).reshape(QT)
    dmf = np.asarray(doc_punct_mask, f32).reshape(ND, LD)
    w1 = np.ascontiguousarray(np.asarray(W1, f32))
    w2 = np.ascontiguousarray(np.asarray(W2, f32))
    b1c = np.ascontiguousarray(np.asarray(b1, f32).reshape(KC, 128).T)
    b2c = np.ascontiguousarray(np.asarray(b2, f32).reshape(E, 1))

    # ---- query compaction ----
    qidx = np.nonzero(qm > 0)[0]
    kq = len(qidx)
    qtp = max(TW, int(-(-kq // TW)) * TW)
    qh_c = np.zeros((qtp, H), f32)
    qh_c[:kq] = qh2[qidx]
    qht = np.ascontiguousarray(qh_c.T)
    qsum = qm.reshape(NQ, LQ).sum(axis=1)
    qsum = np.maximum(qsum, 1.0)
    wind = np.zeros((qtp, NQ), f32)
    qnum = qidx // LQ
    wind[np.arange(kq), qnum] = 1.0 / qsum[qnum]

    # ---- doc compaction ----
    cnt = (dmf > 0).sum(axis=1)
    # every doc keeps >=1 zero slot for the masked-score baseline; the
    # stride is the smallest multiple of 16 that fits (so dtp = 32*ldp
    # stays a multiple of 512); ld=180 bounds it at 192
    ldp = max(32, 16 * int(-(-(int(cnt.max()) + 1) // 16)))
    dtp = NDC * ldp
    ntd = dtp // TW

    in_maps = []
    for c in range(NCORES):
        dh_c = np.zeros((dtp, H), f32)
        # pads carry 1e38 in the ssq accumulator so 1/sqrt() -> ~0 and
        # padded score columns vanish without any mask multiply
        dp_c = np.full(dtp, 1e38, f32)
        for i in range(NDC):
            d = c * NDC + i
            idx = np.nonzero(dmf[d] > 0)[0]
            n = len(idx)
            dh_c[i * ldp:i * ldp + n] = dh2[d * LD + idx]
            dp_c[i * ldp:i * ldp + n] = 0.0
        in_maps.append({
            "dht": np.ascontiguousarray(dh_c.T),
            "qht": qht,
            "w1": w1,
            "w2": w2,
            "b1c": b1c,
            "b2c": b2c,
            "dpad": dp_c.reshape(1, dtp),
            "wind": wind,
        })
    nqr = max(1, int(-(-kq // 128)))
    return in_maps, qtp, ldp, nqr


def kernel(query_hidden, doc_hidden, query_mask, doc_punct_mask,
           W1, b1, W2, b2):
    in_maps, qtp, ldp, nqr = _prep_inputs(query_hidden, doc_hidden,
                                          query_mask, doc_punct_mask,
                                          W1, b1, W2, b2)
    nc = _get_module(qtp, ldp, nqr)
    res = run_bass_kernel_spmd(nc, in_maps, list(range(NCORES)))
    _CACHE["last_results"] = res
    outs = [np.asarray(res.results[c]["out"]) for c in range(NCORES)]
    return np.concatenate(outs, axis=1).astype(np.float32)


# revision 19
# speedup vs baseline: 1.1987x; 1.1181x over previous
"""ColBERT MaxSim retrieval kernel for 8 Trainium2 NeuronCores.

Problem (full shapes):
  query_hidden [64,32,768], doc_hidden [256,180,768], query_mask [64,32],
  doc_punct_mask [256,180], W1 [768,768], b1 [768], W2 [768,128], b2 [128]
  out [64, 256]:
    qe = l2norm(relu(qh@W1+b1)@W2+b2 * qm)        # [64,32,128]
    de = l2norm(relu(dh@W1+b1)@W2+b2 * dm)        # [256,180,128]
    s  = einsum('qih,djh->qidj', qe, de) * dm
    out = s.max(-1).sum(1) / qm.sum(-1, keepdims=True)

Sharding: docs split across the 8 cores (32 docs each); queries are
replicated.  Embarrassingly parallel - no collectives.

Host-side mask compaction (exact, not approximate):
  - only unmasked query tokens are shipped; the query mask and 1/qm.sum()
    normalizer live in a host-built indicator matrix (wind) used as lhsT
    of the final reduction matmul, so padded tokens carry weight 0.  Only
    the ceil(kq/128) query chunks with any unmasked token are ever scored.
  - each doc keeps its unmasked tokens at a fixed stride ldp = smallest
    multiple of 16 >= max_doc_len+1 (the spare slot keeps the reference's
    masked-score 0 baseline under the max).  Pad slots carry 1e38 in a
    host-built row that is accumulated into the sum-of-squares, so
    1/sqrt() makes their embedding columns ~0 with no mask multiply.

Math rearrangement (exactly equivalent up to fp rounding):
  e_masked_normed = e_raw * (mask / max(||e_raw||, eps))  per token
  -> de^T columns scaled by rd = dm/max(||e_raw||,eps) and qe^T columns
     by rq = 1/max(||e_raw||,eps); rq >= 0 commutes with the max over doc
     tokens so scaling qe is exactly the reference math.  The eps clamp is
     fused into the norm sqrt as sqrt(ssq + eps^2) (bitwise-equal outside
     (0, ~eps)).

Static slot schedule: emission is planned over ~1.3us PE "slots" (one W1
column block each).  Per 512-token tile: 6 W1 blocks + W2 block feed the
PE back to back; the per-token sum-of-squares matmul (ones-column lhsT)
is deferred one slot into the next tile so the PE never stalls on the
Act->Pool squares chain; the norm reciprocal lands directly in a [1,*]
scale row (no relay); a ones-row K=1 matmul broadcasts it across
partitions; and the score matmul + segmented max-reduce pairs of the
previous doc range are drained two per slot into the gaps.  That keeps
the DVE-bound MaxSim reductions hidden under tensor-engine head work.
The output reduction matmuls interleave behind the last column block's
reduces.

All embeddings are produced directly in transposed [E/H on partitions,
tokens free] layout so every matmul contraction lands on the partition
dim with zero on-chip transposes (activations are transposed host-side).
fp32r (full-rate fp32 PE path) is used for all matmuls; tile widths
never drop below 256 so fp32r streams at full rate (the last query tile
narrows to exactly 256 when that covers the scored chunks).

TimelineSim cost model: ~134.6us/core (the same model read ~219.9us for
the 224.3us-measured baseline this kernel replaces).
"""

import os
import sys

import numpy as np

for _p in ("/opt/trn_rl_repo",):
    if _p not in sys.path and os.path.isdir(_p):
        sys.path.append(_p)

import concourse.bass as bass
import concourse.mybir as mybir
import concourse.tile as tile
from concourse.bass_utils import run_bass_kernel_spmd

F32 = mybir.dt.float32
F32R = mybir.dt.float32r

# problem dims
NQ, LQ, ND, LD, H, E = 64, 32, 256, 180, 768, 128
NCORES = 8
QT = NQ * LQ                 # 2048 query tokens total (pre-compaction)
NDC = ND // NCORES           # 32 docs per core
KC = H // 128                # 6 contraction chunks
TW = 512                     # token tile width (queries and docs)
GS = 2                       # doc tiles per norm/score pipeline group
EPS = 1e-12

_CACHE = {}


def _build_module(qtp, ldp, nqr, split_waits=True, repeats=1):
    """qtp: padded compacted query-token count (multiple of 512).
    ldp: per-doc token stride after compaction (2*ldp <= 512).
    nqr: number of 128-token query chunks with any unmasked token."""
    dtp = NDC * ldp              # doc tokens per core
    nc = bass.Bass("TRN2", target_bir_lowering=False, debug=False,
                   num_devices=NCORES)

    ntd = dtp // TW
    dh = nc.dram_tensor("dht", [H, dtp], F32R, kind="ExternalInput").ap()
    qh = nc.dram_tensor("qht", [H, qtp], F32R, kind="ExternalInput").ap()
    w1 = nc.dram_tensor("w1", [H, H], F32R, kind="ExternalInput").ap()
    w2 = nc.dram_tensor("w2", [H, E], F32R, kind="ExternalInput").ap()
    b1 = nc.dram_tensor("b1c", [128, KC], F32, kind="ExternalInput").ap()
    b2 = nc.dram_tensor("b2c", [128, 1], F32, kind="ExternalInput").ap()
    dpad = nc.dram_tensor("dpad", [1, dtp], F32R, kind="ExternalInput").ap()
    wind = nc.dram_tensor("wind", [qtp, NQ], F32, kind="ExternalInput").ap()
    out = nc.dram_tensor("out", [NQ, NDC], F32, kind="ExternalOutput").ap()

    with tile.TileContext(nc) as tc:
        for _ in range(repeats):
            _emit(tc, nc, qtp, ldp, nqr, dh, qh, w1, w2, b1, b2, dpad, wind,
                  out)
    if split_waits:
        _split_multi_waits(nc)
    return nc


def _split_multi_waits(nc, max_waits=1):
    """This walrus build rejects instructions carrying more than one sync
    wait (e.g. the S3_LW stage of fused 4-byte matmuls, Drain). Hoist extra
    waits into standalone same-engine InstEventSemaphore instructions placed
    immediately before the offender - semantics are identical since each
    engine executes its stream in order."""
    n = 0
    for f in nc.m.functions:
        for bb in f.blocks:
            new = []
            for ins in bb.instructions:
                si = ins.sync_info
                waits = list(si.on_wait) if si is not None and si.on_wait else []
                if len(waits) > max_waits:
                    for sw in waits[:-max_waits]:
                        n += 1
                        new.append(mybir.InstEventSemaphore(
                            name=f"WS-{n}", engine=ins.engine, ins=[], outs=[],
                            sync_info=mybir.SyncInfo(on_wait=[sw], on_update=[])))
                    ins.sync_info = mybir.SyncInfo(
                        on_wait=waits[-max_waits:],
                        on_update=list(si.on_update) if si.on_update else [])
                new.append(ins)
            bb.instructions = new


def _emit(tc, nc, qtp, ldp, nqr, dh, qh, w1, w2, b1, b2, dpad, wind, out):
    from collections import defaultdict
    from contextlib import ExitStack

    dtp = NDC * ldp
    ntd, ntq = dtp // TW, qtp // TW
    nqch = qtp // 128            # 128-token query chunks
    # score-tile width: as many whole docs as fit a 512-wide psum bank
    dpg = max(1, 512 // ldp)     # docs per score tile (4 @ ldp=112)
    dg = dpg * ldp
    ndg = NDC // dpg

    with ExitStack() as ctx:
        cp = ctx.enter_context(tc.tile_pool(name="consts", bufs=1))
        w1_sb = cp.tile([128, KC, H], F32R, tag="w1sb")
        w2_sb = cp.tile([128, KC, E], F32R, tag="w2sb")
        b1_sb = cp.tile([128, KC], F32, tag="b1sb")
        b2_sb = cp.tile([128, 1], F32, tag="b2sb")
        dpad_sb = cp.tile([1, dtp], F32R, tag="dpadsb")
        wind_sb = cp.tile([128, nqch, NQ], F32, tag="windsb")
        ones_row = cp.tile([1, 128], F32R, tag="onesrow")
        ones_col = cp.tile([128, 1], F32R, tag="onescol")
        deT = cp.tile([128, dtp], F32R, tag="deT")
        qeT = cp.tile([128, qtp], F32R, tag="qeT")
        rd_row = cp.tile([1, dtp], F32R, tag="rdrow")
        rq_row = cp.tile([1, qtp], F32R, tag="rqrow")
        out_sb = cp.tile([NQ, NDC], F32, tag="outsb")

        # w1 split by output-column block so the first W1 matmul only
        # waits on a sixth of the weight traffic
        w1r = w1.rearrange("(k p) h -> p k h", p=128)
        nc.sync.dma_start(out=w1_sb[:, :, 0:128], in_=w1r[:, :, 0:128])

        def dma_consts_rest():
            nc.sync.dma_start(out=b1_sb[:], in_=b1)
            for h in range(1, KC):
                nc.sync.dma_start(out=w1_sb[:, :, h * 128:(h + 1) * 128],
                                  in_=w1r[:, :, h * 128:(h + 1) * 128])
            nc.sync.dma_start(out=w2_sb[:],
                              in_=w2.rearrange("(k p) e -> p k e", p=128))
            nc.sync.dma_start(out=b2_sb[:], in_=b2)

        def dma_consts_late():
            nc.sync.dma_start(out=dpad_sb[:], in_=dpad)
            nc.sync.dma_start(out=wind_sb[:],
                              in_=wind.rearrange("(g p) q -> p g q", p=128))
        # memset can't target f32r; build in f32 scratch and copy (the
        # tensor_copy converts, which satisfies the fp32r rounding rule)
        osc = cp.tile([128, 128], F32, tag="osc")
        nc.vector.memset(osc[:], 1.0)
        nc.vector.tensor_copy(ones_row[:], osc[0:1, :])
        nc.vector.tensor_copy(ones_col[:], osc[:, 0:1])
        eps_sb = cp.tile([16, 1], F32, tag="epsb")
        nc.vector.memset(eps_sb[:], EPS * EPS)

        io_pool = ctx.enter_context(tc.tile_pool(name="io", bufs=5))
        h1_pool = ctx.enter_context(tc.tile_pool(name="h1", bufs=3))
        sq_pool = ctx.enter_context(tc.tile_pool(name="sq", bufs=2))
        nr_pool = ctx.enter_context(tc.tile_pool(name="nr", bufs=2))
        m_pool = ctx.enter_context(tc.tile_pool(name="m", bufs=nqr))

        ph_pool = ctx.enter_context(
            tc.tile_pool(name="ph", bufs=2, space="PSUM"))
        pe_pool = ctx.enter_context(
            tc.tile_pool(name="pex", bufs=2, space="PSUM"))
        psq_pool = ctx.enter_context(
            tc.tile_pool(name="psq", bufs=1, space="PSUM"))
        ps_pool = ctx.enter_context(
            tc.tile_pool(name="ps", bufs=2, space="PSUM"))
        po_pool = ctx.enter_context(
            tc.tile_pool(name="po", bufs=1, space="PSUM"))

        m_tiles = [m_pool.tile([128, NDC], F32, tag="mt", name=f"mt{g}")
                   for g in range(nqr)]
        pout = po_pool.tile([NQ, NDC], F32, tag="pout")
        box = {}

        # ---------------- emission pieces ----------------

        def dma_xt(T, src, base, w):
            xt = io_pool.tile([128, KC, w], F32R, tag="xt", name=f"xt{T}")
            nc.sync.dma_start(out=xt[:],
                              in_=src.rearrange("(k p) n -> p k n", p=128)
                              [:, :, base:base + w])
            box["xt", T] = xt

        def blk_h(T, h, w):
            """One 128-column block of the W1 matmul + biased ReLU."""
            if h == 0:
                box["h1", T] = h1_pool.tile([128, KC, w], F32R, tag="h1",
                                            name=f"h1_{T}")
            h1 = box["h1", T]
            xt = box["xt", T]
            ph = ph_pool.tile([128, w], F32, tag="ph", name="ph")
            for k in range(KC):
                nc.tensor.matmul(
                    ph[:],
                    w1_sb[:, k, h * 128:(h + 1) * 128],
                    xt[:, k, :],
                    start=(k == 0), stop=(k == KC - 1))
            nc.scalar.activation(h1[:, h, :], ph[:],
                                 mybir.ActivationFunctionType.Relu,
                                 bias=b1_sb[:, h:h + 1])

        def blk_w2(T, et_dst, w):
            """W2 matmul + bias into e^T slice, then squares on Pool."""
            h1 = box["h1", T]
            pe = pe_pool.tile([128, w], F32, tag="pe", name="pe")
            for h in range(KC):
                nc.tensor.matmul(pe[:], w2_sb[:, h, :], h1[:, h, :],
                                 start=(h == 0), stop=(h == KC - 1))
            nc.scalar.activation(et_dst, pe[:],
                                 mybir.ActivationFunctionType.Identity,
                                 bias=b2_sb[:, 0:1])
            sq = sq_pool.tile([128, w], F32R, tag="sq", name=f"sq{T}")
            nc.gpsimd.tensor_mul(sq[:], et_dst, et_dst)
            box["sq", T] = sq

        def psq_mm(T, pad_slice):
            """Deferred per-token sum-of-squares row (ones-column lhsT
            contracts the partition dim): scheduled one slot into the NEXT
            tile so the PE never stalls on the Act->Pool sq chain.  For doc
            tiles a second matmul accumulates the host-built pad row (1e38
            at padded slots) so the norm reciprocal vanishes there - no
            mask multiply anywhere."""
            sq = box["sq", T]
            w = sq.shape[-1]
            psq = psq_pool.tile([1, w], F32, tag="psq", name="psq")
            box["psq", T] = psq
            nc.tensor.matmul(psq[:], ones_col[:], sq[:],
                             start=True, stop=pad_slice is None)
            if pad_slice is not None:
                nc.tensor.matmul(psq[:], ones_col[0:1, 0:1], pad_slice,
                                 start=False, stop=True)

        def norm_chain(T, r_row, base):
            """r = 1/sqrt(ssq + EPS^2) written straight into the [1, TW]
            slice of the scale row - no relay needed at one-tile grain.
            sqrt(x+EPS^2) == max(sqrt(x), EPS) exactly at x=0 and for all
            fp32 x >= EPS^2 (1e-24 vanishes in the mantissa); only tokens
            with 0 < ||e|| < ~1e-12 differ, and those scores are ~1e-12."""
            psq = box["psq", T]
            w = psq.shape[-1]
            md = nr_pool.tile([1, w], F32, tag="md", name="md")
            nc.scalar.activation(md[:], psq[:],
                                 mybir.ActivationFunctionType.Sqrt,
                                 bias=eps_sb[0:1, 0:1])
            with nc.allow_low_precision(reason="f32r has ample mantissa "
                                        "for unit-scale norm reciprocals"):
                nc.vector.reciprocal(r_row[:, base:base + w], md[:])

        def scale_tile(eT, r_row, base, w):
            """Scale eT columns [base, base+w) by r_row: K=1 ones-row
            matmul broadcasts the [1, w] slice across 128 partitions."""
            bc = pe_pool.tile([128, w], F32, tag="pe", name="bc")
            nc.tensor.matmul(bc[:], ones_row[:],
                             r_row[:, base:base + w],
                             start=True, stop=True)
            sl = eT[:, base:base + w]
            nc.vector.tensor_mul(sl, sl, bc[:])

        def score_pair(j, g):
            ps = ps_pool.tile([128, dg], F32, tag="ps", name="ps")
            nc.tensor.matmul(ps[:], qeT[:, g * 128:(g + 1) * 128],
                             deT[:, j * dg:(j + 1) * dg],
                             start=True, stop=True)
            nc.vector.tensor_reduce(
                m_tiles[g][:, j * dpg:(j + 1) * dpg],
                ps[:].rearrange("p (d j) -> p d j", j=ldp),
                axis=mybir.AxisListType.X, op=mybir.AluOpType.max)
            if j == ndg - 1:
                # interleave the output reduction behind the last column
                # block's reduces
                nc.tensor.matmul(pout[:], wind_sb[:, g, :], m_tiles[g][:],
                                 start=(g == 0), stop=(g == nqr - 1))

        # ---------------- static schedule ----------------
        # 7 slots per tile (6 W1 column blocks + W2), ~1.3us of PE work
        # each.  Deferred psq matmuls / norm chains / column scaling /
        # score+reduce pairs are injected into later slots so every
        # engine's queue stays covered by PE head work.
        P = 7
        inject = defaultdict(list)
        seq = [0]

        def sched(slot, pri, fn):
            inject[slot].append((pri, seq[0], fn))
            seq[0] += 1

        # query tiles cover only the nqr scored chunks; the last tile
        # narrows to 256 when that suffices (fp32r full rate needs >= 256)
        tiles = []
        cov = 0
        need = nqr * 128
        while cov < need:
            w = TW if need - cov > 256 else 256
            tiles.append(("q", cov, w))
            cov += w
        assert cov <= qtp
        # narrow tile first: the opening activation DMA halves, so the
        # PE's first matmul starts ~2us earlier
        tiles.sort(key=lambda t: t[2])
        tiles += [("d", c * TW, TW) for c in range(ntd)]
        for T, (kind, base, w) in enumerate(tiles):
            src = qh if kind == "q" else dh
            eT = qeT if kind == "q" else deT
            et = eT[:, base:base + w]
            sched(P * T, 1,
                  lambda T=T, src=src, base=base, w=w: dma_xt(T, src, base, w))
            if T == 0:
                sched(0, 1, dma_consts_rest)
                sched(2 * P + 4, 1, dma_consts_late)
            for h in range(KC):
                sched(P * T + h, 2, lambda T=T, h=h, w=w: blk_h(T, h, w))
            sched(P * T + KC, 2, lambda T=T, et=et, w=w: blk_w2(T, et, w))

        # per-tile psq -> norm -> scale chains; scores fire once their
        # docs' columns are scaled.  The very last tile's chain is
        # emitted immediately (nothing left to cover it anyway).
        fired = 0
        for T, (kind, base, w) in enumerate(tiles):
            if kind == "q":
                r_row, eT, pad = rq_row, qeT, None
            else:
                r_row, eT = rd_row, deT
                pad = dpad_sb[0:1, base:base + w]
            last = T == len(tiles) - 1
            ns = P * T + KC + 1 if last else P * (T + 1) + 1
            sk = 0 if last else 2
            sched(ns, 0, lambda T=T, pad=pad: psq_mm(T, pad))
            sched(ns, 1, lambda T=T, r_row=r_row, base=base:
                  norm_chain(T, r_row, base))
            sched(ns + sk + 1, 3, lambda eT=eT, r_row=r_row, base=base, w=w:
                  scale_tile(eT, r_row, base, w))
            if kind == "d":
                idx = 0
                while fired < ndg and (fired + 1) * dg <= base + w:
                    for g in range(nqr):
                        sched(ns + sk + 2 + idx // 2, 3,
                              lambda j=fired, g=g: score_pair(j, g))
                        idx += 1
                    fired += 1
        assert fired == ndg

        for s in range(max(inject) + 1):
            for _, _, fn in sorted(inject[s], key=lambda x: (x[0], x[1])):
                fn()

        nc.vector.tensor_copy(out_sb[:], pout[:])
        nc.sync.dma_start(out=out, in_=out_sb[:])


def _get_module(qtp, ldp, nqr):
    key = ("nc", qtp, ldp, nqr)
    if key not in _CACHE:
        _CACHE[key] = _build_module(qtp, ldp, nqr)
    return _CACHE[key]


def _prep_inputs(query_hidden, doc_hidden, query_mask, doc_punct_mask,
                 W1, b1, W2, b2):
    """Host-side compaction + shard + layout prep.
    Returns (per-core input maps, qtp, ldp)."""
    f32 = np.float32
    qh2 = np.asarray(query_hidden, f32).reshape(QT, H)
    dh2 = np.asarray(doc_hidden, f32).reshape(ND * LD, H)
    qm = np.asarray(query_mask, f32).reshape(QT)
    dmf = np.asarray(doc_punct_mask, f32).reshape(ND, LD)
    w1 = np.ascontiguousarray(np.asarray(W1, f32))
    w2 = np.ascontiguousarray(np.asarray(W2, f32))
    b1c = np.ascontiguousarray(np.asarray(b1, f32).reshape(KC, 128).T)
    b2c = np.ascontiguousarray(np.asarray(b2, f32).reshape(E, 1))

    # ---- query compaction ----
    qidx = np.nonzero(qm > 0)[0]
    kq = len(qidx)
    qtp = max(TW, int(-(-kq // TW)) * TW)
    qh_c = np.zeros((qtp, H), f32)
    qh_c[:kq] = qh2[qidx]
    qht = np.ascontiguousarray(qh_c.T)
    qsum = qm.reshape(NQ, LQ).sum(axis=1)
    qsum = np.maximum(qsum, 1.0)
    wind = np.zeros((qtp, NQ), f32)
    qnum = qidx // LQ
    wind[np.arange(kq), qnum] = 1.0 / qsum[qnum]

    # ---- doc compaction ----
    cnt = (dmf > 0).sum(axis=1)
    # every doc keeps >=1 zero slot for the masked-score baseline; the
    # stride is the smallest multiple of 16 that fits (so dtp = 32*ldp
    # stays a multiple of 512); ld=180 bounds it at 192
    ldp = max(32, 16 * int(-(-(int(cnt.max()) + 1) // 16)))
    dtp = NDC * ldp
    ntd = dtp // TW

    in_maps = []
    for c in range(NCORES):
        dh_c = np.zeros((dtp, H), f32)
        # pads carry 1e38 in the ssq accumulator so 1/sqrt() -> ~0 and
        # padded score columns vanish without any mask multiply
        dp_c = np.full(dtp, 1e38, f32)
        for i in range(NDC):
            d = c * NDC + i
            idx = np.nonzero(dmf[d] > 0)[0]
            n = len(idx)
            dh_c[i * ldp:i * ldp + n] = dh2[d * LD + idx]
            dp_c[i * ldp:i * ldp + n] = 0.0
        in_maps.append({
            "dht": np.ascontiguousarray(dh_c.T),
            "qht": qht,
            "w1": w1,
            "w2": w2,
            "b1c": b1c,
            "b2c": b2c,
            "dpad": dp_c.reshape(1, dtp),
            "wind": wind,
        })
    nqr = max(1, int(-(-kq // 128)))
    return in_maps, qtp, ldp, nqr


def kernel(query_hidden, doc_hidden, query_mask, doc_punct_mask,
           W1, b1, W2, b2):
    in_maps, qtp, ldp, nqr = _prep_inputs(query_hidden, doc_hidden,
                                          query_mask, doc_punct_mask,
                                          W1, b1, W2, b2)
    nc = _get_module(qtp, ldp, nqr)
    res = run_bass_kernel_spmd(nc, in_maps, list(range(NCORES)))
    _CACHE["last_results"] = res
    outs = [np.asarray(res.results[c]["out"]) for c in range(NCORES)]
    return np.concatenate(outs, axis=1).astype(np.float32)
